# revision 19
# baseline (speedup 1.0000x reference)
"""Causal self-attention with RoPE on 8 Trainium2 NeuronCores.

Sharding: tensor-parallel over heads x data-parallel over batch.
  core c -> batch b = c // 2, head-group g = c % 2 (heads 8g .. 8g+7).
Each core computes qkv projections for its 8 heads, RoPE, causal
attention, and a *partial* output projection (its heads' contribution
to y[b]). Host sums the two partials per batch and adds the bias
terms (b_proj and the v-bias routed through W_proj).

v4 (over v3):
  - host pre-transposes x / wq / wk / wv into DMA-friendly layouts
    (4-16 KB descriptors instead of 256 B-1 KB) so the first matmul
    starts earlier and DMA triggers are cheap.
  - attention q-chunks run in DESCENDING order (qc3 first): the first
    block, which has no projection filler to interleave, is the one
    whose exp/mask critical chain is mildest; C(qc+1) fills block qc;
    C(0) is the PE-dense tail.
  - softmax denominator finished by gpsimd partition_all_reduce
    (replaces the ones-matmul + [1,512] reciprocal + broadcast) which
    frees a PSUM bank -> psY triple-buffered.
  - all projection-PSUM evictions on DVE (ACT-queue evictions were
    recycling PSUM buffers too late behind the exp backlog).
"""

import numpy as np
import ml_dtypes

import concourse.bass as bass
import concourse.mybir as mybir
import concourse.tile as tile
from concourse import bacc
from concourse.bass_isa import ReduceOp
from concourse.bass_utils import run_bass_kernel_spmd

F32 = mybir.dt.float32
BF16 = mybir.dt.bfloat16
AF = mybir.ActivationFunctionType
ALU = mybir.AluOpType

D_MODEL = 2048
N_HEADS = 16
HD = 128
B, T = 4, 2048
N_CORES = 8
HPC = 8           # heads per core
PB = 128          # partitions / k-chunk
XSL = 512         # x^T t-slice width in phase A
QB = 512          # phase-B query-chunk width (one PSUM bank of f32)
EC = 512          # phase-C output-column chunk width
SCALE = 1.0 / np.sqrt(HD)

BF = ml_dtypes.bfloat16


def build_nc(t=T, d=D_MODEL, hpc=HPC, compile=True):
    """Build the per-core Bass module. All 8 cores run this same module on
    different input slices."""
    nc = bacc.Bacc(trn_type="TRN2", target_bir_lowering=False)

    dck = d // PB          # D-chunks (contraction tiles)
    nsl = t // XSL         # phase-A t-slices
    ntc128 = t // PB       # t-chunks of 128
    hw = hpc * HD          # this core's head width
    nqc = t // QB          # phase-B query chunks
    kpq = QB // PB         # k-blocks per query chunk
    nec = d // EC          # phase-C output-column chunks

    # DMA-friendly input layouts (prepared host-side in make_in_maps):
    #   x8[sl, cc8, p, c8, tt] = x[b].T[(cc8*8+c8)*128 + p, sl*512 + tt]
    #   wq8/wk8[p, h, c, m]    = W[(c*128+p), h*128 + m]
    #   wv8[nci, p, c, m]      = Wv[(c*128+p), nci*512 + m]
    x8 = nc.dram_tensor("x8", [nsl, 4, PB, (dck // 4) * XSL], BF16,
                        kind="ExternalInput")
    wq8 = nc.dram_tensor("wq8", [hpc, PB, dck * HD], BF16,
                         kind="ExternalInput")
    wk8 = nc.dram_tensor("wk8", [hpc, PB, dck * HD], BF16,
                         kind="ExternalInput")
    wv8 = nc.dram_tensor("wv8", [2, PB, dck * 512], BF16,
                         kind="ExternalInput")
    bq = nc.dram_tensor("bq", [hw], F32, kind="ExternalInput")
    bk = nc.dram_tensor("bk", [hw], F32, kind="ExternalInput")
    wp = nc.dram_tensor("wp", [hw, d], BF16, kind="ExternalInput")
    cosT = nc.dram_tensor("cosT", [HD, t], BF16, kind="ExternalInput")
    # sinTr is sign-folded (rows 0:64 of the raw table negated) and then
    # rolled by 64 partitions, so the partition-shifted rope muls read both
    # DVE inputs at the same base partition (walrus requirement).
    sinTr = nc.dram_tensor("sinTr", [HD, t], BF16, kind="ExternalInput")
    y = nc.dram_tensor("y", [t, d], BF16, kind="ExternalOutput")

    NEG_BIG = -1e30

    with tile.TileContext(nc) as tc:
        with (
            tc.tile_pool(name="consts", bufs=1) as consts,
            tc.tile_pool(name="qkres", bufs=1) as qkres,
            tc.tile_pool(name="vall", bufs=1) as pv,
        ):
            # constants for the matmul-side causal mask (used in the first,
            # filler-less attention block): identity and a shifted triangle
            # TRI[p, g] = NEG_BIG iff (g - 384) < p, else 0.  Slicing
            # TRI[:, 384-128r : 896-128r] gives the additive mask for
            # diagonal offset r.
            ones_f = consts.tile([PB, 1], F32, tag="ones_f")
            nc.vector.memset(ones_f, 1.0)
            ones_col = consts.tile([PB, 1], BF16, tag="ones")
            nc.vector.tensor_copy(ones_col, ones_f)

            # warm up the PE HAM clock gate while the first DMAs land:
            # ~50 tiny matmuls on a scratch tile keep the PE "busy" so the
            # first real matmuls run at 2.4 GHz instead of 1.2 GHz.
            warm = consts.tile([PB, 64], BF16, tag="warm")
            nc.vector.memset(warm, 0.0)
            with tc.tile_pool(name="psWarm", bufs=1, space="PSUM") as psW:
                wps = psW.tile([64, 64], F32, tag="wps", name="wps")
                for _ in range(270):
                    nc.tensor.matmul(wps, lhsT=warm[:, 0:64], rhs=warm,
                                     start=True, stop=True)

            # persistent tiles: q^T/k^T (through B) and v (through B)
            qT_all = qkres.tile([HD, hpc, t], BF16, tag="qT")
            kT_all = qkres.tile([HD, hpc, t], BF16, tag="kT")
            v_all = pv.tile([PB, ntc128, hw], BF16, tag="v_all", name="v_all")

            # ============ Phase A1+A2: projections (x fully resident) ======
            with (
                tc.tile_pool(name="x_a", bufs=4) as px,
                tc.tile_pool(name="w_v0", bufs=1) as pwv0,
            ):
                wv_h0 = pwv0.tile([PB, dck, 512], BF16, tag="wv0",
                                  name="wv_h0")
                # ---- A1: q,k per head (weights streamed per head) + RoPE --
                with (
                    tc.tile_pool(name="ropec", bufs=1) as ropec,
                    tc.tile_pool(name="w_qk", bufs=3) as pw,
                    tc.tile_pool(name="t_a1", bufs=2) as pt,
                    tc.tile_pool(name="psA1", bufs=3, space="PSUM") as psA,
                ):
                    cosT_s = ropec.tile([HD, t], BF16, tag="cosT")
                    sinT_s = ropec.tile([HD, t], BF16, tag="sinT")
                    bq_s = ropec.tile([HD, hpc], F32, tag="bq")
                    bk_s = ropec.tile([HD, hpc], F32, tag="bk")

                    w_tiles = {}

                    def load_w(h, eng):
                        for kind, w_src in (("q", wq8), ("k", wk8)):
                            w_h = pw.tile([PB, dck, HD], BF16, tag="w",
                                          name="w_h")
                            eng.dma_start(out=w_h, in_=w_src.ap()[h])
                            w_tiles[(kind, h)] = w_h

                    # Startup-critical DMAs go on ONE queue (sync) in exact
                    # consumption order -- queues share HBM bandwidth, so a
                    # separate queue only gets a fraction.  Biases (tiny,
                    # needed by the first ACT evict) go first on scalar;
                    # the remaining heads' weights stream on scalar behind.
                    nc.scalar.dma_start(
                        out=bq_s, in_=bq.ap().rearrange("(h p) -> p h", p=HD))
                    nc.scalar.dma_start(
                        out=bk_s, in_=bk.ap().rearrange("(h p) -> p h", p=HD))
                    wq_h0 = pw.tile([PB, dck, HD], BF16, tag="w", name="w_h")
                    nc.sync.dma_start(out=wq_h0, in_=wq8.ap()[0])
                    w_tiles[("q", 0)] = wq_h0
                    xt_tiles = []
                    for sl in range(nsl):
                        xt_s = px.tile([PB, dck, XSL], BF16, tag="xt",
                                       name="xt_s")
                        xt_tiles.append(xt_s)
                    nc.sync.dma_start(out=xt_tiles[0][:, 0:4, :],
                                      in_=x8.ap()[0, 0])
                    nc.sync.dma_start(out=xt_tiles[0][:, 4:8, :],
                                      in_=x8.ap()[0, 1])
                    wk_h0 = pw.tile([PB, dck, HD], BF16, tag="w", name="w_h")
                    nc.sync.dma_start(out=wk_h0, in_=wk8.ap()[0])
                    nc.sync.dma_start(out=cosT_s, in_=cosT.ap())
                    nc.sync.dma_start(out=sinT_s, in_=sinTr.ap())
                    w_tiles[("k", 0)] = wk_h0
                    nc.sync.dma_start(out=xt_tiles[0][:, 8:12, :],
                                      in_=x8.ap()[0, 2])
                    nc.sync.dma_start(out=xt_tiles[0][:, 12:16, :],
                                      in_=x8.ap()[0, 3])
                    for sl in range(1, nsl):
                        for cc4 in range(4):
                            nc.sync.dma_start(
                                out=xt_tiles[sl][:, cc4 * 4:(cc4 + 1) * 4, :],
                                in_=x8.ap()[sl, cc4],
                            )

                    for h in range(hpc):
                        for sl in range(nsl):
                            ts = slice(sl * XSL, (sl + 1) * XSL)
                            xt_s = xt_tiles[sl]
                            for kind, bias_s, outT in (
                                ("q", bq_s, qT_all),
                                ("k", bk_s, kT_all),
                            ):
                                w_h = w_tiles[(kind, h)]
                                ps = psA.tile([PB, XSL], F32, tag="ps_a",
                                              name="ps_a")
                                for c in range(dck):
                                    nc.tensor.matmul(
                                        ps,
                                        lhsT=w_h[:, c, :],
                                        rhs=xt_s[:, c, :],
                                        start=(c == 0),
                                        stop=(c == dck - 1),
                                    )
                                # evict + per-partition bias on ACT -> bf16
                                raw = pt.tile([PB, XSL], BF16, tag="raw",
                                              name="raw")
                                nc.scalar.activation(
                                    out=raw, in_=ps, func=AF.Identity,
                                    bias=bias_s[:, h:h + 1], scale=1.0,
                                )
                                # rope on DVE:
                                #   out = raw*cos + shift64(raw)*sin_folded
                                rs = pt.tile([PB, XSL], BF16, tag="rs",
                                             name="rs")
                                nc.vector.tensor_mul(
                                    rs[0:64], raw[64:128],
                                    sinT_s[64:128, ts])
                                nc.vector.tensor_mul(
                                    rs[64:128], raw[0:64], sinT_s[0:64, ts])
                                cq = pt.tile([PB, XSL], BF16, tag="cq",
                                             name="cq")
                                nc.vector.tensor_mul(cq, raw, cosT_s[:, ts])
                                nc.vector.tensor_add(outT[:, h, ts], cq, rs)
                            if sl == 0 and h + 1 < hpc:
                                load_w(h + 1, nc.scalar)
                            if sl == 0 and h == 2:
                                nc.gpsimd.dma_start(out=wv_h0,
                                                    in_=wv8.ap()[0])
                        for kind in ("q", "k"):
                            w_tiles.pop((kind, h))

                # ---- A2: v projection (natural layout, x slices reused) ---
                # nci-outer: the first half of wv covers 16 groups of
                # compute, so the second half streams in fully hidden.
                with (
                    tc.tile_pool(name="w_v1", bufs=1) as pwv1,
                    tc.tile_pool(name="psA2", bufs=3, space="PSUM") as psA,
                ):
                    wv_h1 = pwv1.tile([PB, dck, 512], BF16, tag="wv1",
                                      name="wv_h1")
                    nc.gpsimd.dma_start(out=wv_h1, in_=wv8.ap()[1])
                    for nci, wv_h in ((0, wv_h0), (1, wv_h1)):
                        ns = slice(nci * 512, (nci + 1) * 512)
                        for kcg in range(ntc128):
                            xt_s = xt_tiles[kcg // (XSL // PB)]
                            t128 = kcg % (XSL // PB)
                            ps = psA.tile([PB, 512], F32, tag="ps_v",
                                          name="ps_v")
                            for c in range(dck):
                                nc.tensor.matmul(
                                    ps,
                                    lhsT=xt_s[:, c,
                                              t128 * PB:(t128 + 1) * PB],
                                    rhs=wv_h[:, c, :],
                                    start=(c == 0),
                                    stop=(c == dck - 1),
                                )
                            nc.vector.tensor_copy(v_all[:, kcg, ns], ps)

            # ========== Phase B+C: attention fused with projection =========
            with (
                tc.tile_pool(name="ot", bufs=1) as po,
                tc.tile_pool(name="wp_p", bufs=1) as pwp,
                tc.tile_pool(name="bconsts", bufs=1) as bconsts,
            ):
                # mask constants (gpsimd affine_select used only here, at
                # the phase boundary: swapping gpsimd custom-op libraries
                # mid-loop costs ~us per swap).
                # ident/tri: matmul-side causal mask for the first block.
                # masks[r][p, f] = 1.0 iff f >= p + 128*r for the rest.
                ident = bconsts.tile([PB, PB], BF16, tag="ident")
                nc.vector.memset(ident, 1.0)
                nc.gpsimd.affine_select(
                    out=ident, in_=ident, compare_op=ALU.is_ge, fill=0.0,
                    base=0, pattern=[[1, PB]], channel_multiplier=-1)
                nc.gpsimd.affine_select(
                    out=ident, in_=ident, compare_op=ALU.is_ge, fill=0.0,
                    base=0, pattern=[[-1, PB]], channel_multiplier=1)
                tri = bconsts.tile([PB, 896], BF16, tag="tri")
                nc.vector.memset(tri, 0.0)
                nc.gpsimd.affine_select(
                    out=tri, in_=tri, compare_op=ALU.is_ge, fill=NEG_BIG,
                    base=-384, pattern=[[1, 896]], channel_multiplier=-1)
                masks = []
                for r in range(QB // PB):
                    mk_f = bconsts.tile([PB, QB], F32, tag="mask_f",
                                        name="mask_f")
                    nc.vector.memset(mk_f, 1.0)
                    nc.gpsimd.affine_select(
                        out=mk_f, in_=mk_f, compare_op=ALU.is_ge, fill=0.0,
                        base=-(r * PB), pattern=[[1, QB]],
                        channel_multiplier=-1,
                    )
                    mk = bconsts.tile([PB, QB], BF16, tag=f"mask{r}",
                                      name=f"mask{r}")
                    nc.vector.tensor_copy(mk, mk_f)
                    masks.append(mk)

                ot_all = po.tile([HD, hpc, t], BF16, tag="ot")
                # prefetch the output-projection weights on the gpsimd queue
                wp_s = pwp.tile([PB, hpc, d], BF16, tag="wp", name="wp_s")
                wp_src = wp.ap().rearrange("(h p) e -> p h e", p=PB)
                for h in range(hpc):
                    nc.gpsimd.dma_start(out=wp_s[:, h, :],
                                        in_=wp_src[:, h, :])
                with (
                    tc.tile_pool(name="pt_pool", bufs=6) as pp,
                    tc.tile_pool(name="zac", bufs=2) as pza,
                    tc.tile_pool(name="small", bufs=2) as psm,
                    tc.tile_pool(name="yout", bufs=3) as py,
                    tc.tile_pool(name="psS", bufs=2, space="PSUM") as psS,
                    tc.tile_pool(name="psO", bufs=1, space="PSUM") as psO,
                    tc.tile_pool(name="psZ", bufs=1, space="PSUM") as psZ,
                    tc.tile_pool(name="psY", bufs=2, space="PSUM") as psY,
                ):
                    # attention q-chunks in DESCENDING order, head-inner;
                    # the projection for q-chunk qc+1 interleaves with the
                    # attention pairs of q-chunk qc.
                    qcs = list(range(nqc - 1, -1, -1))
                    pairs = []
                    block_start = {}
                    for qc in qcs:
                        block_start[qc] = len(pairs)
                        for h in range(hpc):
                            for kcp in range((qc + 1) * kpq // 2):
                                pairs.append((qc, h, kcp))
                    st = {}

                    def emit_S(i):
                        qc, h, kcp = pairs[i]
                        if kcp == 0:
                            st[(qc, h, "o")] = psO.tile(
                                [HD, QB], F32, tag="ps_o", name="ps_o")
                            st[(qc, h, "z")] = pza.tile(
                                [PB, 2 * QB], BF16, tag="z_acc",
                                name="z_acc")
                        ps_s2 = psS.tile([PB, 2 * QB], F32, tag="ps_s",
                                         name="ps_s2")
                        qs = slice(qc * QB, (qc + 1) * QB)
                        for j in (0, 1):
                            kc = 2 * kcp + j
                            r = kc - qc * kpq
                            # first (filler-less) block: fold the causal
                            # mask into the S accumulation with a -BIG
                            # triangle matmul, so exp needs no gpsimd hop
                            mask_mm = (qc == qcs[0] and r >= 0)
                            nc.tensor.matmul(
                                ps_s2[:, j * QB:(j + 1) * QB],
                                lhsT=kT_all[:, h, kc * PB:(kc + 1) * PB],
                                rhs=qT_all[:, h, qs],
                                start=True, stop=not mask_mm,
                            )
                            if mask_mm:
                                nc.tensor.matmul(
                                    ps_s2[:, j * QB:(j + 1) * QB],
                                    lhsT=ident,
                                    rhs=tri[:, 384 - 128 * r:
                                            896 - 128 * r],
                                    start=False, stop=True,
                                )
                        st[i] = ps_s2

                    def emit_expmask(i):
                        qc, h, kcp = pairs[i]
                        ps_s2 = st.pop(i)
                        pt2 = pp.tile([PB, 2 * QB], BF16, tag="pt",
                                      name="pt2")
                        nc.scalar.activation(
                            out=pt2, in_=ps_s2, func=AF.Exp, scale=SCALE,
                        )
                        # causal mask: multiply diagonal blocks by the
                        # precomputed mask on DVE (the first block is
                        # already masked via the triangle matmul)
                        for j in (0, 1):
                            r = 2 * kcp + j - qc * kpq
                            if r >= 0 and qc != qcs[0]:
                                nc.vector.tensor_mul(
                                    pt2[:, j * QB:(j + 1) * QB],
                                    pt2[:, j * QB:(j + 1) * QB],
                                    masks[r],
                                )
                        st[(i, "pt")] = pt2

                    def emit_zo(i):
                        qc, h, kcp = pairs[i]
                        nkc = (qc + 1) * kpq
                        pt2 = st.pop((i, "pt"))
                        z_acc = st[(qc, h, "z")]
                        # softmax denominator: accumulate exp tiles on DVE
                        if kcp == 0:
                            nc.vector.tensor_copy(z_acc, pt2)
                        else:
                            nc.vector.tensor_add(z_acc, z_acc, pt2)
                        ps_o = st[(qc, h, "o")]
                        for j in (0, 1):
                            kc = 2 * kcp + j
                            nc.tensor.matmul(
                                ps_o,
                                lhsT=v_all[:, kc, h * HD:(h + 1) * HD],
                                rhs=pt2[:, j * QB:(j + 1) * QB],
                                start=(kc == 0), stop=(kc == nkc - 1),
                            )
                        if 2 * kcp + 1 == nkc - 1:
                            # head (h, qc) done: evict the O accumulator on
                            # ACT (frees the psO bank fast), then finish the
                            # softmax denominator in two deferred stages so
                            # no engine queue ever waits cross-engine:
                            #   stage 1 (next pair): PE column-sums z_acc
                            #     (the DVE z-adds have drained by then)
                            #   stage 2 (pair after): DVE reciprocal, then
                            #     gpsimd broadcast + normalize multiply
                            ot_tmp = pp.tile([HD, QB], BF16,
                                             tag="ot_tmp", name="ot_tmp",
                                             bufs=2)
                            nc.scalar.copy(ot_tmp, st.pop((qc, h, "o")))
                            z_fin = st.pop((qc, h, "z"))

                            def stage1(z_acc=z_fin, qc=qc, h=h):
                                ps_z = psZ.tile([1, QB], F32, tag="ps_z",
                                                name="ps_z")
                                for j in (0, 1):
                                    nc.tensor.matmul(
                                        ps_z,
                                        lhsT=ones_col,
                                        rhs=z_acc[:, j * QB:(j + 1) * QB],
                                        start=(j == 0), stop=(j == 1),
                                    )
                                st[(qc, h, "pz")] = ps_z

                            def stage2(qc=qc, h=h):
                                ps_z = st.pop((qc, h, "pz"))
                                rz = psm.tile([1, QB], F32, tag="rz",
                                              name="rz")
                                nc.vector.reciprocal_approx_fast(
                                    out=rz, in_=ps_z)
                                rzb = pp.tile([HD, QB], F32, tag="rzb",
                                              name="rzb", bufs=2)
                                nc.gpsimd.partition_broadcast(rzb, rz)
                                st[(qc, h, "rzb")] = rzb

                            def stage3(qc=qc, h=h, ot_tmp=ot_tmp):
                                qs = slice(qc * QB, (qc + 1) * QB)
                                nc.vector.tensor_mul(
                                    ot_all[:, h, qs], ot_tmp,
                                    st.pop((qc, h, "rzb")))

                            pending.append((i + 1, stage1))
                            pending.append((i + 2, stage2))
                            pending.append((i + 3, stage3))

                    def emit_c_group(cqc, ui):
                        # half-group units: 4 head-matmuls per injection
                        gi, half = ui // 2, ui % 2
                        t128 = cqc * kpq + gi // nec
                        nci = gi % nec
                        es = slice(nci * EC, (nci + 1) * EC)
                        if nci == 0 and half == 0:
                            st[("y", t128)] = py.tile(
                                [PB, d], BF16, tag="y_t", name="y_t")
                        y_t = st[("y", t128)]
                        if half == 0:
                            st[("py", t128, nci)] = psY.tile(
                                [PB, EC], F32, tag="ps_y", name="ps_y")
                        ps_y = st[("py", t128, nci)]
                        for h in range(half * 4, half * 4 + 4):
                            nc.tensor.matmul(
                                ps_y,
                                lhsT=ot_all[:, h,
                                            t128 * PB:(t128 + 1) * PB],
                                rhs=wp_s[:, h, es],
                                start=(h == 0), stop=(h == hpc - 1),
                            )
                        if half != 1:
                            return
                        st.pop(("py", t128, nci))
                        nc.vector.tensor_copy(y_t[:, es], ps_y)
                        if cqc == qcs[-1]:
                            # final block: stream each column chunk so the
                            # last DMA before teardown is small
                            nc.sync.dma_start(
                                out=y.ap()[t128 * PB:(t128 + 1) * PB, es],
                                in_=y_t[:, es],
                            )
                            if nci == nec - 1:
                                st.pop(("y", t128))
                        elif nci == nec - 1:
                            nc.sync.dma_start(
                                out=y.ap()[t128 * PB:(t128 + 1) * PB, :],
                                in_=st.pop(("y", t128)),
                            )

                    ngroups = kpq * nec * 2  # C half-groups per q-chunk
                    emitted_c = {qc: 0 for qc in range(nqc)}
                    pending = []
                    emit_S(0)
                    emit_expmask(0)
                    emit_S(1)
                    for i in range(len(pairs)):
                        if i + 2 < len(pairs):
                            emit_S(i + 2)
                        if i + 1 < len(pairs):
                            emit_expmask(i + 1)
                        for due, fn in [p for p in pending if p[0] <= i]:
                            pending.remove((due, fn))
                            fn()
                        emit_zo(i)
                        qc = pairs[i][0]
                        bi = qcs.index(qc)
                        if bi >= 1:
                            cqc = qcs[bi - 1]   # project the previous block
                            j = i - block_start[qc]
                            npq = hpc * (qc + 1) * kpq // 2
                            # hold injection until the previous block's
                            # deferred finalize stages have been emitted
                            target = ((j - 3) * ngroups // npq
                                      if j >= 4 else 0)
                            while emitted_c[cqc] < target:
                                emit_c_group(cqc, emitted_c[cqc])
                                emitted_c[cqc] += 1
                    for due, fn in pending:
                        fn()
                    pending = []
                    for qc in qcs[:-1]:
                        while emitted_c[qc] < ngroups:
                            emit_c_group(qc, emitted_c[qc])
                            emitted_c[qc] += 1
                    for gi in range(ngroups):
                        emit_c_group(qcs[-1], gi)
    if compile:
        nc.compile()
    return nc


def make_in_maps(x, cos, sin, W_qkv, b_qkv, W_proj):
    """Host-side sharding: build the 8 per-core input dicts (bf16 casts)."""
    d = x.shape[-1]
    dck = d // PB
    in_maps = []
    cosT = np.ascontiguousarray(cos.reshape(-1, HD).T).astype(np.float32)
    sinT = np.ascontiguousarray(sin.reshape(-1, HD).T).astype(np.float32)
    sinTs = sinT.copy()
    sinTs[: HD // 2] = -sinTs[: HD // 2]
    sinTr = np.roll(sinTs, -(HD // 2), axis=0)
    cosT = cosT.astype(BF)
    sinTr = sinTr.astype(BF)
    Wq = np.asarray(W_qkv[:, 0 * d:1 * d], np.float32)
    Wk = np.asarray(W_qkv[:, 1 * d:2 * d], np.float32)
    Wv = np.asarray(W_qkv[:, 2 * d:3 * d], np.float32)

    def wqk8(Wm):
        # [d, hw] -> [h, p, c*m]  (contiguous per head for fast DMA)
        return np.ascontiguousarray(
            Wm.reshape(dck, PB, HPC, HD).transpose(2, 1, 0, 3)
        ).astype(BF).reshape(HPC, PB, dck * HD)

    def wv8(Wm):
        # [d, hw] -> [nci, p, c*m]
        return np.ascontiguousarray(
            Wm.reshape(dck, PB, 2, 512).transpose(2, 1, 0, 3)
        ).astype(BF).reshape(2, PB, dck * 512)

    def x8(xb):
        # x[b] [t, d] -> x.T [d, t] -> [sl, cc4, p, c4*tt]
        xT = np.asarray(xb, np.float32).T
        return np.ascontiguousarray(
            xT.reshape(4, 4, PB, 4, XSL).transpose(3, 0, 2, 1, 4)
        ).astype(BF).reshape(4, 4, PB, 4 * XSL)

    for c in range(N_CORES):
        b = c // 2
        g = c % 2
        hw = HPC * HD
        cs = slice(g * hw, (g + 1) * hw)
        in_maps.append(
            {
                "x8": x8(x[b]),
                "wq8": wqk8(Wq[:, cs]),
                "wk8": wqk8(Wk[:, cs]),
                "wv8": wv8(Wv[:, cs]),
                "bq": np.ascontiguousarray(b_qkv[0 * d:1 * d][cs], np.float32),
                "bk": np.ascontiguousarray(b_qkv[1 * d:2 * d][cs], np.float32),
                "wp": np.ascontiguousarray(
                    np.asarray(W_proj, np.float32)[g * hw:(g + 1) * hw, :]
                ).astype(BF),
                "cosT": cosT,
                "sinTr": sinTr,
            }
        )
    return in_maps


def gather_output(results, b_qkv, W_proj, b_proj):
    """Sum the per-core partials and add the bias terms."""
    d = W_proj.shape[1]
    # v-bias contributes (sum_k attn = 1) exactly b_v @ W_proj per token.
    host_bias = (
        np.asarray(b_qkv[2 * d:3 * d], np.float32)
        @ np.asarray(W_proj, np.float32)
        + np.asarray(b_proj, np.float32)
    )
    y = np.empty((B, T, d), np.float32)
    for b in range(B):
        y[b] = (np.asarray(results[2 * b]["y"], np.float32)
                + np.asarray(results[2 * b + 1]["y"], np.float32)
                + host_bias)
    return y


_NC_CACHE = {}


def kernel(x, cos, sin, W_qkv, b_qkv, W_proj, b_proj):
    x = np.asarray(x, np.float32)
    key = "full"
    if key not in _NC_CACHE:
        _NC_CACHE[key] = build_nc()
    nc = _NC_CACHE[key]
    in_maps = make_in_maps(
        x,
        np.asarray(cos, np.float32),
        np.asarray(sin, np.float32),
        np.asarray(W_qkv, np.float32),
        np.asarray(b_qkv, np.float32),
        np.asarray(W_proj, np.float32),
    )
    res = run_bass_kernel_spmd(nc, in_maps, core_ids=list(range(N_CORES)))
    return gather_output(res.results, b_qkv, W_proj, b_proj)


if __name__ == "__main__":
    import reference

    inputs = reference.setup_inputs()
    out = kernel(**{k: np.asarray(v) for k, v in inputs.items()})
    exp = np.asarray(reference.reference(**inputs))
    err = np.abs(out - exp).max() / np.abs(exp).max()
    print("rel err:", err)


# revision 20
# speedup vs baseline: 1.0012x; 1.0012x over previous
"""Causal self-attention with RoPE on 8 Trainium2 NeuronCores.

Sharding: tensor-parallel over heads x data-parallel over batch.
  core c -> batch b = c // 2, head-group g = c % 2 (heads 8g .. 8g+7).
Each core computes qkv projections for its 8 heads, RoPE, causal
attention, and a *partial* output projection (its heads' contribution
to y[b]). Host sums the two partials per batch and adds the bias
terms (b_proj and the v-bias routed through W_proj).

v4 (over v3):
  - host pre-transposes x / wq / wk / wv into DMA-friendly layouts
    (4-16 KB descriptors instead of 256 B-1 KB) so the first matmul
    starts earlier and DMA triggers are cheap.
  - attention q-chunks run in DESCENDING order (qc3 first): the first
    block, which has no projection filler to interleave, is the one
    whose exp/mask critical chain is mildest; C(qc+1) fills block qc;
    C(0) is the PE-dense tail.
  - softmax denominator finished by gpsimd partition_all_reduce
    (replaces the ones-matmul + [1,512] reciprocal + broadcast) which
    frees a PSUM bank -> psY triple-buffered.
  - all projection-PSUM evictions on DVE (ACT-queue evictions were
    recycling PSUM buffers too late behind the exp backlog).
"""

import numpy as np
import ml_dtypes

import concourse.bass as bass
import concourse.mybir as mybir
import concourse.tile as tile
from concourse import bacc
from concourse.bass_isa import ReduceOp
from concourse.bass_utils import run_bass_kernel_spmd

F32 = mybir.dt.float32
BF16 = mybir.dt.bfloat16
AF = mybir.ActivationFunctionType
ALU = mybir.AluOpType

D_MODEL = 2048
N_HEADS = 16
HD = 128
B, T = 4, 2048
N_CORES = 8
HPC = 8           # heads per core
PB = 128          # partitions / k-chunk
XSL = 512         # x^T t-slice width in phase A
QB = 512          # phase-B query-chunk width (one PSUM bank of f32)
EC = 512          # phase-C output-column chunk width
SCALE = 1.0 / np.sqrt(HD)

BF = ml_dtypes.bfloat16


def build_nc(t=T, d=D_MODEL, hpc=HPC, compile=True):
    """Build the per-core Bass module. All 8 cores run this same module on
    different input slices."""
    nc = bacc.Bacc(trn_type="TRN2", target_bir_lowering=False)

    dck = d // PB          # D-chunks (contraction tiles)
    nsl = t // XSL         # phase-A t-slices
    ntc128 = t // PB       # t-chunks of 128
    hw = hpc * HD          # this core's head width
    nqc = t // QB          # phase-B query chunks
    kpq = QB // PB         # k-blocks per query chunk
    nec = d // EC          # phase-C output-column chunks

    # DMA-friendly input layouts (prepared host-side in make_in_maps):
    #   x8[sl, cc8, p, c8, tt] = x[b].T[(cc8*8+c8)*128 + p, sl*512 + tt]
    #   wq8/wk8[p, h, c, m]    = W[(c*128+p), h*128 + m]
    #   wv8[nci, p, c, m]      = Wv[(c*128+p), nci*512 + m]
    x8 = nc.dram_tensor("x8", [nsl, 2, PB, (dck // 2) * XSL], BF16,
                        kind="ExternalInput")
    wq8 = nc.dram_tensor("wq8", [hpc, PB, dck * HD], BF16,
                         kind="ExternalInput")
    wk8 = nc.dram_tensor("wk8", [hpc, PB, dck * HD], BF16,
                         kind="ExternalInput")
    wv8 = nc.dram_tensor("wv8", [2, PB, dck * 512], BF16,
                         kind="ExternalInput")
    bq = nc.dram_tensor("bq", [hw], F32, kind="ExternalInput")
    bk = nc.dram_tensor("bk", [hw], F32, kind="ExternalInput")
    wp = nc.dram_tensor("wp", [hw, d], BF16, kind="ExternalInput")
    cosT = nc.dram_tensor("cosT", [HD, t], BF16, kind="ExternalInput")
    # sinTr is sign-folded (rows 0:64 of the raw table negated) and then
    # rolled by 64 partitions, so the partition-shifted rope muls read both
    # DVE inputs at the same base partition (walrus requirement).
    sinTr = nc.dram_tensor("sinTr", [HD, t], BF16, kind="ExternalInput")
    y = nc.dram_tensor("y", [t, d], BF16, kind="ExternalOutput")

    NEG_BIG = -1e30

    with tile.TileContext(nc) as tc:
        with (
            tc.tile_pool(name="consts", bufs=1) as consts,
            tc.tile_pool(name="qkres", bufs=1) as qkres,
            tc.tile_pool(name="vall", bufs=1) as pv,
        ):
            # constants for the matmul-side causal mask (used in the first,
            # filler-less attention block): identity and a shifted triangle
            # TRI[p, g] = NEG_BIG iff (g - 384) < p, else 0.  Slicing
            # TRI[:, 384-128r : 896-128r] gives the additive mask for
            # diagonal offset r.
            ones_f = consts.tile([PB, 1], F32, tag="ones_f")
            nc.vector.memset(ones_f, 1.0)
            ones_col = consts.tile([PB, 1], BF16, tag="ones")
            nc.vector.tensor_copy(ones_col, ones_f)

            # warm up the PE HAM clock gate while the first DMAs land:
            # tiny matmuls on a scratch tile keep the PE "busy" so the
            # first real matmuls run at 2.4 GHz instead of 1.2 GHz.
            warm = consts.tile([PB, 64], BF16, tag="warm")
            nc.vector.memset(warm, 0.0)
            with tc.tile_pool(name="psWarm", bufs=1, space="PSUM") as psW:
                wps = psW.tile([64, 64], F32, tag="wps", name="wps")
                for _ in range(270):
                    nc.tensor.matmul(wps, lhsT=warm[:, 0:64], rhs=warm,
                                     start=True, stop=True)

            # persistent tiles: q^T/k^T (through B) and v (through B)
            qT_all = qkres.tile([HD, hpc, t], BF16, tag="qT")
            kT_all = qkres.tile([HD, hpc, t], BF16, tag="kT")
            v_all = pv.tile([PB, ntc128, hw], BF16, tag="v_all", name="v_all")

            # ============ Phase A1+A2: projections (x fully resident) ======
            with (
                tc.tile_pool(name="x_a", bufs=4) as px,
                tc.tile_pool(name="w_v0", bufs=1) as pwv0,
            ):
                wv_h0 = pwv0.tile([PB, dck, 512], BF16, tag="wv0",
                                  name="wv_h0")
                # ---- A1: q,k per head (weights streamed per head) + RoPE --
                with (
                    tc.tile_pool(name="ropec", bufs=1) as ropec,
                    tc.tile_pool(name="w_qk", bufs=3) as pw,
                    tc.tile_pool(name="t_a1", bufs=2) as pt,
                    tc.tile_pool(name="psA1", bufs=3, space="PSUM") as psA,
                ):
                    cosT_s = ropec.tile([HD, t], BF16, tag="cosT")
                    sinT_s = ropec.tile([HD, t], BF16, tag="sinT")
                    bq_s = ropec.tile([HD, hpc], F32, tag="bq")
                    bk_s = ropec.tile([HD, hpc], F32, tag="bk")

                    w_tiles = {}

                    def load_w(h, eng):
                        for kind, w_src in (("q", wq8), ("k", wk8)):
                            w_h = pw.tile([PB, dck, HD], BF16, tag="w",
                                          name="w_h")
                            eng.dma_start(out=w_h, in_=w_src.ap()[h])
                            w_tiles[(kind, h)] = w_h

                    # Startup-critical DMAs go on ONE queue (sync) in exact
                    # consumption order -- queues share HBM bandwidth, so a
                    # separate queue only gets a fraction.  Biases (tiny,
                    # needed by the first ACT evict) go first on scalar;
                    # the remaining heads' weights stream on scalar behind.
                    nc.scalar.dma_start(
                        out=bq_s, in_=bq.ap().rearrange("(h p) -> p h", p=HD))
                    nc.scalar.dma_start(
                        out=bk_s, in_=bk.ap().rearrange("(h p) -> p h", p=HD))
                    wq_h0 = pw.tile([PB, dck, HD], BF16, tag="w", name="w_h")
                    nc.sync.dma_start(out=wq_h0, in_=wq8.ap()[0])
                    w_tiles[("q", 0)] = wq_h0
                    xt_tiles = []
                    for sl in range(nsl):
                        xt_s = px.tile([PB, dck, XSL], BF16, tag="xt",
                                       name="xt_s")
                        xt_tiles.append(xt_s)
                    nc.sync.dma_start(out=xt_tiles[0][:, 0:8, :],
                                      in_=x8.ap()[0, 0])
                    wk_h0 = pw.tile([PB, dck, HD], BF16, tag="w", name="w_h")
                    nc.sync.dma_start(out=wk_h0, in_=wk8.ap()[0])
                    w_tiles[("k", 0)] = wk_h0
                    nc.sync.dma_start(out=cosT_s, in_=cosT.ap())
                    nc.sync.dma_start(out=sinT_s, in_=sinTr.ap())
                    nc.sync.dma_start(out=xt_tiles[0][:, 8:16, :],
                                      in_=x8.ap()[0, 1])
                    for sl in range(1, nsl):
                        for cc8 in range(2):
                            nc.sync.dma_start(
                                out=xt_tiles[sl][:, cc8 * 8:(cc8 + 1) * 8, :],
                                in_=x8.ap()[sl, cc8],
                            )

                    for h in range(hpc):
                        for sl in range(nsl):
                            ts = slice(sl * XSL, (sl + 1) * XSL)
                            xt_s = xt_tiles[sl]
                            for kind, bias_s, outT in (
                                ("q", bq_s, qT_all),
                                ("k", bk_s, kT_all),
                            ):
                                w_h = w_tiles[(kind, h)]
                                ps = psA.tile([PB, XSL], F32, tag="ps_a",
                                              name="ps_a")
                                for c in range(dck):
                                    nc.tensor.matmul(
                                        ps,
                                        lhsT=w_h[:, c, :],
                                        rhs=xt_s[:, c, :],
                                        start=(c == 0),
                                        stop=(c == dck - 1),
                                    )
                                # evict + per-partition bias on ACT -> bf16
                                raw = pt.tile([PB, XSL], BF16, tag="raw",
                                              name="raw")
                                nc.scalar.activation(
                                    out=raw, in_=ps, func=AF.Identity,
                                    bias=bias_s[:, h:h + 1], scale=1.0,
                                )
                                # rope on DVE:
                                #   out = raw*cos + shift64(raw)*sin_folded
                                rs = pt.tile([PB, XSL], BF16, tag="rs",
                                             name="rs")
                                nc.vector.tensor_mul(
                                    rs[0:64], raw[64:128],
                                    sinT_s[64:128, ts])
                                nc.vector.tensor_mul(
                                    rs[64:128], raw[0:64], sinT_s[0:64, ts])
                                cq = pt.tile([PB, XSL], BF16, tag="cq",
                                             name="cq")
                                nc.vector.tensor_mul(cq, raw, cosT_s[:, ts])
                                nc.vector.tensor_add(outT[:, h, ts], cq, rs)
                            if sl == 0 and h + 1 < hpc:
                                load_w(h + 1, nc.scalar)
                            if sl == 0 and h == 2:
                                nc.gpsimd.dma_start(out=wv_h0,
                                                    in_=wv8.ap()[0])
                        for kind in ("q", "k"):
                            w_tiles.pop((kind, h))

                # ---- A2: v projection (natural layout, x slices reused) ---
                # nci-outer: the first half of wv covers 16 groups of
                # compute, so the second half streams in fully hidden.
                with (
                    tc.tile_pool(name="w_v1", bufs=1) as pwv1,
                    tc.tile_pool(name="psA2", bufs=3, space="PSUM") as psA,
                ):
                    wv_h1 = pwv1.tile([PB, dck, 512], BF16, tag="wv1",
                                      name="wv_h1")
                    nc.gpsimd.dma_start(out=wv_h1, in_=wv8.ap()[1])
                    for nci, wv_h in ((0, wv_h0), (1, wv_h1)):
                        ns = slice(nci * 512, (nci + 1) * 512)
                        for kcg in range(ntc128):
                            xt_s = xt_tiles[kcg // (XSL // PB)]
                            t128 = kcg % (XSL // PB)
                            ps = psA.tile([PB, 512], F32, tag="ps_v",
                                          name="ps_v")
                            for c in range(dck):
                                nc.tensor.matmul(
                                    ps,
                                    lhsT=xt_s[:, c,
                                              t128 * PB:(t128 + 1) * PB],
                                    rhs=wv_h[:, c, :],
                                    start=(c == 0),
                                    stop=(c == dck - 1),
                                )
                            nc.vector.tensor_copy(v_all[:, kcg, ns], ps)

            # ========== Phase B+C: attention fused with projection =========
            with (
                tc.tile_pool(name="ot", bufs=1) as po,
                tc.tile_pool(name="wp_p", bufs=1) as pwp,
                tc.tile_pool(name="bconsts", bufs=1) as bconsts,
            ):
                # mask constants (gpsimd affine_select used only here, at
                # the phase boundary: swapping gpsimd custom-op libraries
                # mid-loop costs ~us per swap).
                # ident/tri: matmul-side causal mask for the first block.
                # masks[r][p, f] = 1.0 iff f >= p + 128*r for the rest.
                ident = bconsts.tile([PB, PB], BF16, tag="ident")
                nc.vector.memset(ident, 1.0)
                nc.gpsimd.affine_select(
                    out=ident, in_=ident, compare_op=ALU.is_ge, fill=0.0,
                    base=0, pattern=[[1, PB]], channel_multiplier=-1)
                nc.gpsimd.affine_select(
                    out=ident, in_=ident, compare_op=ALU.is_ge, fill=0.0,
                    base=0, pattern=[[-1, PB]], channel_multiplier=1)
                tri = bconsts.tile([PB, 896], BF16, tag="tri")
                nc.vector.memset(tri, 0.0)
                nc.gpsimd.affine_select(
                    out=tri, in_=tri, compare_op=ALU.is_ge, fill=NEG_BIG,
                    base=-384, pattern=[[1, 896]], channel_multiplier=-1)
                masks = []
                for r in range(QB // PB):
                    mk_f = bconsts.tile([PB, QB], F32, tag="mask_f",
                                        name="mask_f")
                    nc.vector.memset(mk_f, 1.0)
                    nc.gpsimd.affine_select(
                        out=mk_f, in_=mk_f, compare_op=ALU.is_ge, fill=0.0,
                        base=-(r * PB), pattern=[[1, QB]],
                        channel_multiplier=-1,
                    )
                    mk = bconsts.tile([PB, QB], BF16, tag=f"mask{r}",
                                      name=f"mask{r}")
                    nc.vector.tensor_copy(mk, mk_f)
                    masks.append(mk)

                ot_all = po.tile([HD, hpc, t], BF16, tag="ot")
                # prefetch the output-projection weights on the gpsimd queue
                wp_s = pwp.tile([PB, hpc, d], BF16, tag="wp", name="wp_s")
                wp_src = wp.ap().rearrange("(h p) e -> p h e", p=PB)
                for h in range(hpc):
                    nc.gpsimd.dma_start(out=wp_s[:, h, :],
                                        in_=wp_src[:, h, :])
                with (
                    tc.tile_pool(name="pt_pool", bufs=6) as pp,
                    tc.tile_pool(name="zac", bufs=2) as pza,
                    tc.tile_pool(name="small", bufs=2) as psm,
                    tc.tile_pool(name="yout", bufs=3) as py,
                    tc.tile_pool(name="psS", bufs=2, space="PSUM") as psS,
                    tc.tile_pool(name="psO", bufs=1, space="PSUM") as psO,
                    tc.tile_pool(name="psZ", bufs=1, space="PSUM") as psZ,
                    tc.tile_pool(name="psY", bufs=2, space="PSUM") as psY,
                ):
                    # attention q-chunks in DESCENDING order, head-inner;
                    # the projection for q-chunk qc+1 interleaves with the
                    # attention pairs of q-chunk qc.
                    qcs = list(range(nqc - 1, -1, -1))
                    pairs = []
                    block_start = {}
                    for qc in qcs:
                        block_start[qc] = len(pairs)
                        for h in range(hpc):
                            for kcp in range((qc + 1) * kpq // 2):
                                pairs.append((qc, h, kcp))
                    st = {}

                    def emit_S(i):
                        qc, h, kcp = pairs[i]
                        if kcp == 0:
                            st[(qc, h, "o")] = psO.tile(
                                [HD, QB], F32, tag="ps_o", name="ps_o")
                            st[(qc, h, "z")] = pza.tile(
                                [PB, 2 * QB], BF16, tag="z_acc",
                                name="z_acc")
                        ps_s2 = psS.tile([PB, 2 * QB], F32, tag="ps_s",
                                         name="ps_s2")
                        qs = slice(qc * QB, (qc + 1) * QB)
                        for j in (0, 1):
                            kc = 2 * kcp + j
                            r = kc - qc * kpq
                            # first (filler-less) block: fold the causal
                            # mask into the S accumulation with a -BIG
                            # triangle matmul, so exp needs no gpsimd hop
                            mask_mm = (qc == qcs[0] and r >= 0)
                            nc.tensor.matmul(
                                ps_s2[:, j * QB:(j + 1) * QB],
                                lhsT=kT_all[:, h, kc * PB:(kc + 1) * PB],
                                rhs=qT_all[:, h, qs],
                                start=True, stop=not mask_mm,
                            )
                            if mask_mm:
                                nc.tensor.matmul(
                                    ps_s2[:, j * QB:(j + 1) * QB],
                                    lhsT=ident,
                                    rhs=tri[:, 384 - 128 * r:
                                            896 - 128 * r],
                                    start=False, stop=True,
                                )
                        st[i] = ps_s2

                    def emit_expmask(i):
                        qc, h, kcp = pairs[i]
                        ps_s2 = st.pop(i)
                        pt2 = pp.tile([PB, 2 * QB], BF16, tag="pt",
                                      name="pt2")
                        nc.scalar.activation(
                            out=pt2, in_=ps_s2, func=AF.Exp, scale=SCALE,
                        )
                        # causal mask: multiply diagonal blocks by the
                        # precomputed mask on DVE (the first block is
                        # already masked via the triangle matmul)
                        for j in (0, 1):
                            r = 2 * kcp + j - qc * kpq
                            if r >= 0 and qc != qcs[0]:
                                nc.vector.tensor_mul(
                                    pt2[:, j * QB:(j + 1) * QB],
                                    pt2[:, j * QB:(j + 1) * QB],
                                    masks[r],
                                )
                        st[(i, "pt")] = pt2

                    def emit_zo(i):
                        qc, h, kcp = pairs[i]
                        nkc = (qc + 1) * kpq
                        pt2 = st.pop((i, "pt"))
                        z_acc = st[(qc, h, "z")]
                        # softmax denominator: accumulate exp tiles on DVE
                        if kcp == 0:
                            nc.vector.tensor_copy(z_acc, pt2)
                        else:
                            nc.vector.tensor_add(z_acc, z_acc, pt2)
                        ps_o = st[(qc, h, "o")]
                        for j in (0, 1):
                            kc = 2 * kcp + j
                            nc.tensor.matmul(
                                ps_o,
                                lhsT=v_all[:, kc, h * HD:(h + 1) * HD],
                                rhs=pt2[:, j * QB:(j + 1) * QB],
                                start=(kc == 0), stop=(kc == nkc - 1),
                            )
                        if 2 * kcp + 1 == nkc - 1:
                            # head (h, qc) done: evict the O accumulator on
                            # ACT (frees the psO bank fast), then finish the
                            # softmax denominator in two deferred stages so
                            # no engine queue ever waits cross-engine:
                            #   stage 1 (next pair): PE column-sums z_acc
                            #     (the DVE z-adds have drained by then)
                            #   stage 2 (pair after): DVE reciprocal, then
                            #     gpsimd broadcast + normalize multiply
                            ot_tmp = pp.tile([HD, QB], BF16,
                                             tag="ot_tmp", name="ot_tmp",
                                             bufs=2)
                            nc.scalar.copy(ot_tmp, st.pop((qc, h, "o")))
                            z_fin = st.pop((qc, h, "z"))

                            def stage1(z_acc=z_fin, qc=qc, h=h):
                                ps_z = psZ.tile([1, QB], F32, tag="ps_z",
                                                name="ps_z")
                                for j in (0, 1):
                                    nc.tensor.matmul(
                                        ps_z,
                                        lhsT=ones_col,
                                        rhs=z_acc[:, j * QB:(j + 1) * QB],
                                        start=(j == 0), stop=(j == 1),
                                    )
                                st[(qc, h, "pz")] = ps_z

                            def stage2(qc=qc, h=h):
                                ps_z = st.pop((qc, h, "pz"))
                                rz = psm.tile([1, QB], F32, tag="rz",
                                              name="rz")
                                nc.vector.reciprocal_approx_fast(
                                    out=rz, in_=ps_z)
                                rzb = pp.tile([HD, QB], F32, tag="rzb",
                                              name="rzb", bufs=2)
                                nc.gpsimd.partition_broadcast(rzb, rz)
                                st[(qc, h, "rzb")] = rzb

                            def stage3(qc=qc, h=h, ot_tmp=ot_tmp):
                                qs = slice(qc * QB, (qc + 1) * QB)
                                nc.vector.tensor_mul(
                                    ot_all[:, h, qs], ot_tmp,
                                    st.pop((qc, h, "rzb")))

                            pending.append((i + 1, stage1))
                            pending.append((i + 2, stage2))
                            pending.append((i + 3, stage3))

                    def emit_c_group(cqc, gi):
                        t128 = cqc * kpq + gi // nec
                        nci = gi % nec
                        es = slice(nci * EC, (nci + 1) * EC)
                        if nci == 0:
                            st[("y", t128)] = py.tile(
                                [PB, d], BF16, tag="y_t", name="y_t")
                        y_t = st[("y", t128)]
                        ps_y = psY.tile([PB, EC], F32, tag="ps_y",
                                        name="ps_y")
                        for h in range(hpc):
                            nc.tensor.matmul(
                                ps_y,
                                lhsT=ot_all[:, h,
                                            t128 * PB:(t128 + 1) * PB],
                                rhs=wp_s[:, h, es],
                                start=(h == 0), stop=(h == hpc - 1),
                            )
                        nc.vector.tensor_copy(y_t[:, es], ps_y)
                        if cqc == qcs[-1]:
                            # final block: stream each column chunk so the
                            # last DMA before teardown is small
                            nc.sync.dma_start(
                                out=y.ap()[t128 * PB:(t128 + 1) * PB, es],
                                in_=y_t[:, es],
                            )
                            if nci == nec - 1:
                                st.pop(("y", t128))
                        elif nci == nec - 1:
                            nc.sync.dma_start(
                                out=y.ap()[t128 * PB:(t128 + 1) * PB, :],
                                in_=st.pop(("y", t128)),
                            )

                    ngroups = kpq * nec      # C matmul groups per q-chunk
                    emitted_c = {qc: 0 for qc in range(nqc)}
                    pending = []
                    emit_S(0)
                    emit_expmask(0)
                    emit_S(1)
                    for i in range(len(pairs)):
                        if i + 2 < len(pairs):
                            emit_S(i + 2)
                        if i + 1 < len(pairs):
                            emit_expmask(i + 1)
                        for due, fn in [p for p in pending if p[0] <= i]:
                            pending.remove((due, fn))
                            fn()
                        emit_zo(i)
                        qc = pairs[i][0]
                        bi = qcs.index(qc)
                        if bi >= 1:
                            cqc = qcs[bi - 1]   # project the previous block
                            j = i - block_start[qc]
                            npq = hpc * (qc + 1) * kpq // 2
                            # hold injection until the previous block's
                            # deferred finalize stages have been emitted
                            target = ((j - 3) * ngroups // npq
                                      if j >= 4 else 0)
                            while emitted_c[cqc] < target:
                                emit_c_group(cqc, emitted_c[cqc])
                                emitted_c[cqc] += 1
                    for due, fn in pending:
                        fn()
                    pending = []
                    for qc in qcs[:-1]:
                        while emitted_c[qc] < ngroups:
                            emit_c_group(qc, emitted_c[qc])
                            emitted_c[qc] += 1
                    for gi in range(ngroups):
                        emit_c_group(qcs[-1], gi)
    if compile:
        nc.compile()
    return nc


def make_in_maps(x, cos, sin, W_qkv, b_qkv, W_proj):
    """Host-side sharding: build the 8 per-core input dicts (bf16 casts)."""
    d = x.shape[-1]
    dck = d // PB
    in_maps = []
    cosT = np.ascontiguousarray(cos.reshape(-1, HD).T).astype(np.float32)
    sinT = np.ascontiguousarray(sin.reshape(-1, HD).T).astype(np.float32)
    sinTs = sinT.copy()
    sinTs[: HD // 2] = -sinTs[: HD // 2]
    sinTr = np.roll(sinTs, -(HD // 2), axis=0)
    cosT = cosT.astype(BF)
    sinTr = sinTr.astype(BF)
    Wq = np.asarray(W_qkv[:, 0 * d:1 * d], np.float32)
    Wk = np.asarray(W_qkv[:, 1 * d:2 * d], np.float32)
    Wv = np.asarray(W_qkv[:, 2 * d:3 * d], np.float32)

    def wqk8(Wm):
        # [d, hw] -> [h, p, c*m]  (contiguous per head for fast DMA)
        return np.ascontiguousarray(
            Wm.reshape(dck, PB, HPC, HD).transpose(2, 1, 0, 3)
        ).astype(BF).reshape(HPC, PB, dck * HD)

    def wv8(Wm):
        # [d, hw] -> [nci, p, c*m]
        return np.ascontiguousarray(
            Wm.reshape(dck, PB, 2, 512).transpose(2, 1, 0, 3)
        ).astype(BF).reshape(2, PB, dck * 512)

    def x8(xb):
        # x[b] [t, d] -> x.T [d, t] -> [sl, cc8, p, c8*tt]
        xT = np.asarray(xb, np.float32).T
        return np.ascontiguousarray(
            xT.reshape(2, 8, PB, 4, XSL).transpose(3, 0, 2, 1, 4)
        ).astype(BF).reshape(4, 2, PB, 8 * XSL)

    for c in range(N_CORES):
        b = c // 2
        g = c % 2
        hw = HPC * HD
        cs = slice(g * hw, (g + 1) * hw)
        in_maps.append(
            {
                "x8": x8(x[b]),
                "wq8": wqk8(Wq[:, cs]),
                "wk8": wqk8(Wk[:, cs]),
                "wv8": wv8(Wv[:, cs]),
                "bq": np.ascontiguousarray(b_qkv[0 * d:1 * d][cs], np.float32),
                "bk": np.ascontiguousarray(b_qkv[1 * d:2 * d][cs], np.float32),
                "wp": np.ascontiguousarray(
                    np.asarray(W_proj, np.float32)[g * hw:(g + 1) * hw, :]
                ).astype(BF),
                "cosT": cosT,
                "sinTr": sinTr,
            }
        )
    return in_maps


def gather_output(results, b_qkv, W_proj, b_proj):
    """Sum the per-core partials and add the bias terms."""
    d = W_proj.shape[1]
    # v-bias contributes (sum_k attn = 1) exactly b_v @ W_proj per token.
    host_bias = (
        np.asarray(b_qkv[2 * d:3 * d], np.float32)
        @ np.asarray(W_proj, np.float32)
        + np.asarray(b_proj, np.float32)
    )
    y = np.empty((B, T, d), np.float32)
    for b in range(B):
        y[b] = (np.asarray(results[2 * b]["y"], np.float32)
                + np.asarray(results[2 * b + 1]["y"], np.float32)
                + host_bias)
    return y


_NC_CACHE = {}


def kernel(x, cos, sin, W_qkv, b_qkv, W_proj, b_proj):
    x = np.asarray(x, np.float32)
    key = "full"
    if key not in _NC_CACHE:
        _NC_CACHE[key] = build_nc()
    nc = _NC_CACHE[key]
    in_maps = make_in_maps(
        x,
        np.asarray(cos, np.float32),
        np.asarray(sin, np.float32),
        np.asarray(W_qkv, np.float32),
        np.asarray(b_qkv, np.float32),
        np.asarray(W_proj, np.float32),
    )
    res = run_bass_kernel_spmd(nc, in_maps, core_ids=list(range(N_CORES)))
    return gather_output(res.results, b_qkv, W_proj, b_proj)


if __name__ == "__main__":
    import reference

    inputs = reference.setup_inputs()
    out = kernel(**{k: np.asarray(v) for k, v in inputs.items()})
    exp = np.asarray(reference.reference(**inputs))
    err = np.abs(out - exp).max() / np.abs(exp).max()
    print("rel err:", err)


# revision 21
# speedup vs baseline: 1.0082x; 1.0070x over previous
"""Causal self-attention with RoPE on 8 Trainium2 NeuronCores.

Sharding: tensor-parallel over heads x data-parallel over batch.
  core c -> batch b = c // 2, head-group g = c % 2 (heads 8g .. 8g+7).
Each core computes qkv projections for its 8 heads, RoPE, causal
attention, and a *partial* output projection (its heads' contribution
to y[b]). Host sums the two partials per batch and adds the bias
terms (b_proj and the v-bias routed through W_proj).

Optimizations over the phase-serial baseline (741us -> ~674us):
  - softmax denominator off the PE: DVE accumulates the exp tiles
    (z_acc += pt2 per pair), one ones-matmul per (head, q-chunk)
    finishes the column sum (saves ~280 PE matmuls = ~60us).
  - phase C (output projection) fused into phase B: attention runs
    q-chunk DESCENDING / head-inner, and the projection matmul groups
    for the previous q-chunk interleave between attention pairs, so
    the PE stays busy while ACT computes exp (ACT exp 153us is slower
    than attention's PE work alone).
  - the first block (qc3, no filler available) folds its causal mask
    into the S accumulation with a -1e30 triangle matmul; the other
    blocks use DVE mask multiplies.  gpsimd runs ONLY
    partition_broadcast in the loop: mixing gpsimd custom-op types
    (affine_select / broadcast / reduce) reloads a 27KB microcode
    library per switch (~us each).
  - softmax finalize staged across pairs (ACT evicts ps_o -> PE
    z-matmuls at +1 pair -> DVE reciprocal + gpsimd broadcast at +2 ->
    DVE normalize mul at +3) so no engine FIFO ever blocks waiting on
    another engine's in-flight work.
  - host pre-transposes x / wq / wk / wv into per-DMA-contiguous
    layouts (4-16KB descriptors); startup-critical transfers go on one
    queue in consumption order (queues share HBM bandwidth).
"""

import numpy as np
import ml_dtypes

import concourse.bass as bass
import concourse.mybir as mybir
import concourse.tile as tile
from concourse import bacc
from concourse.bass_isa import ReduceOp
from concourse.bass_utils import run_bass_kernel_spmd

F32 = mybir.dt.float32
BF16 = mybir.dt.bfloat16
AF = mybir.ActivationFunctionType
ALU = mybir.AluOpType

D_MODEL = 2048
N_HEADS = 16
HD = 128
B, T = 4, 2048
N_CORES = 8
HPC = 8           # heads per core
PB = 128          # partitions / k-chunk
XSL = 512         # x^T t-slice width in phase A
QB = 512          # phase-B query-chunk width (one PSUM bank of f32)
EC = 512          # phase-C output-column chunk width
SCALE = 1.0 / np.sqrt(HD)

BF = ml_dtypes.bfloat16


def build_nc(t=T, d=D_MODEL, hpc=HPC, compile=True):
    """Build the per-core Bass module. All 8 cores run this same module on
    different input slices."""
    nc = bacc.Bacc(trn_type="TRN2", target_bir_lowering=False)

    dck = d // PB          # D-chunks (contraction tiles)
    nsl = t // XSL         # phase-A t-slices
    ntc128 = t // PB       # t-chunks of 128
    hw = hpc * HD          # this core's head width
    nqc = t // QB          # phase-B query chunks
    kpq = QB // PB         # k-blocks per query chunk
    nec = d // EC          # phase-C output-column chunks

    # DMA-friendly input layouts (prepared host-side in make_in_maps):
    #   x8[sl, cc8, p, c8, tt] = x[b].T[(cc8*8+c8)*128 + p, sl*512 + tt]
    #   wq8/wk8[p, h, c, m]    = W[(c*128+p), h*128 + m]
    #   wv8[nci, p, c, m]      = Wv[(c*128+p), nci*512 + m]
    x8 = nc.dram_tensor("x8", [nsl, 2, PB, (dck // 2) * XSL], BF16,
                        kind="ExternalInput")
    wq8 = nc.dram_tensor("wq8", [hpc, PB, dck * HD], BF16,
                         kind="ExternalInput")
    wk8 = nc.dram_tensor("wk8", [hpc, PB, dck * HD], BF16,
                         kind="ExternalInput")
    wv8 = nc.dram_tensor("wv8", [2, PB, dck * 512], BF16,
                         kind="ExternalInput")
    bq = nc.dram_tensor("bq", [hw], F32, kind="ExternalInput")
    bk = nc.dram_tensor("bk", [hw], F32, kind="ExternalInput")
    wp = nc.dram_tensor("wp", [hw, d], BF16, kind="ExternalInput")
    cosT = nc.dram_tensor("cosT", [HD, t], BF16, kind="ExternalInput")
    # sinTr is sign-folded (rows 0:64 of the raw table negated) and then
    # rolled by 64 partitions, so the partition-shifted rope muls read both
    # DVE inputs at the same base partition (walrus requirement).
    sinTr = nc.dram_tensor("sinTr", [HD, t], BF16, kind="ExternalInput")
    y = nc.dram_tensor("y", [t, d], BF16, kind="ExternalOutput")

    NEG_BIG = -1e30

    with tile.TileContext(nc) as tc:
        with (
            tc.tile_pool(name="consts", bufs=1) as consts,
            tc.tile_pool(name="qkres", bufs=1) as qkres,
            tc.tile_pool(name="vall", bufs=1) as pv,
        ):
            # constants for the matmul-side causal mask (used in the first,
            # filler-less attention block): identity and a shifted triangle
            # TRI[p, g] = NEG_BIG iff (g - 384) < p, else 0.  Slicing
            # TRI[:, 384-128r : 896-128r] gives the additive mask for
            # diagonal offset r.
            ones_f = consts.tile([PB, 1], F32, tag="ones_f")
            nc.vector.memset(ones_f, 1.0)
            ones_col = consts.tile([PB, 1], BF16, tag="ones")
            nc.vector.tensor_copy(ones_col, ones_f)

            # persistent tiles: q^T/k^T (through B) and v (through B)
            qT_all = qkres.tile([HD, hpc, t], BF16, tag="qT")
            kT_all = qkres.tile([HD, hpc, t], BF16, tag="kT")
            v_all = pv.tile([PB, ntc128, hw], BF16, tag="v_all", name="v_all")

            # ============ Phase A1+A2: projections (x fully resident) ======
            with (
                tc.tile_pool(name="x_a", bufs=4) as px,
                tc.tile_pool(name="w_v0", bufs=1) as pwv0,
            ):
                wv_h0 = pwv0.tile([PB, dck, 512], BF16, tag="wv0",
                                  name="wv_h0")
                # ---- A1: q,k per head (weights streamed per head) + RoPE --
                with (
                    tc.tile_pool(name="ropec", bufs=1) as ropec,
                    tc.tile_pool(name="w_qk", bufs=3) as pw,
                    tc.tile_pool(name="t_a1", bufs=2) as pt,
                    tc.tile_pool(name="psA1", bufs=3, space="PSUM") as psA,
                ):
                    cosT_s = ropec.tile([HD, t], BF16, tag="cosT")
                    sinT_s = ropec.tile([HD, t], BF16, tag="sinT")
                    bq_s = ropec.tile([HD, hpc], F32, tag="bq")
                    bk_s = ropec.tile([HD, hpc], F32, tag="bk")

                    w_tiles = {}

                    def load_w(h, eng):
                        for kind, w_src in (("q", wq8), ("k", wk8)):
                            w_h = pw.tile([PB, dck, HD], BF16, tag="w",
                                          name="w_h")
                            eng.dma_start(out=w_h, in_=w_src.ap()[h])
                            w_tiles[(kind, h)] = w_h

                    # Startup-critical DMAs go on ONE queue (sync) in exact
                    # consumption order -- queues share HBM bandwidth, so a
                    # separate queue only gets a fraction.  Biases (tiny,
                    # needed by the first ACT evict) go first on scalar;
                    # the remaining heads' weights stream on scalar behind.
                    nc.scalar.dma_start(
                        out=bq_s, in_=bq.ap().rearrange("(h p) -> p h", p=HD))
                    nc.scalar.dma_start(
                        out=bk_s, in_=bk.ap().rearrange("(h p) -> p h", p=HD))
                    wq_h0 = pw.tile([PB, dck, HD], BF16, tag="w", name="w_h")
                    nc.sync.dma_start(out=wq_h0, in_=wq8.ap()[0])
                    w_tiles[("q", 0)] = wq_h0
                    xt_tiles = []
                    for sl in range(nsl):
                        xt_s = px.tile([PB, dck, XSL], BF16, tag="xt",
                                       name="xt_s")
                        xt_tiles.append(xt_s)
                    nc.sync.dma_start(out=xt_tiles[0][:, 0:8, :],
                                      in_=x8.ap()[0, 0])
                    nc.sync.dma_start(out=xt_tiles[0][:, 8:16, :],
                                      in_=x8.ap()[0, 1])
                    wk_h0 = pw.tile([PB, dck, HD], BF16, tag="w", name="w_h")
                    nc.sync.dma_start(out=wk_h0, in_=wk8.ap()[0])
                    w_tiles[("k", 0)] = wk_h0
                    nc.sync.dma_start(out=cosT_s, in_=cosT.ap())
                    nc.sync.dma_start(out=sinT_s, in_=sinTr.ap())
                    for sl in range(1, nsl):
                        for cc8 in range(2):
                            nc.sync.dma_start(
                                out=xt_tiles[sl][:, cc8 * 8:(cc8 + 1) * 8, :],
                                in_=x8.ap()[sl, cc8],
                            )

                    for h in range(hpc):
                        for sl in range(nsl):
                            ts = slice(sl * XSL, (sl + 1) * XSL)
                            xt_s = xt_tiles[sl]
                            for kind, bias_s, outT in (
                                ("q", bq_s, qT_all),
                                ("k", bk_s, kT_all),
                            ):
                                w_h = w_tiles[(kind, h)]
                                ps = psA.tile([PB, XSL], F32, tag="ps_a",
                                              name="ps_a")
                                for c in range(dck):
                                    nc.tensor.matmul(
                                        ps,
                                        lhsT=w_h[:, c, :],
                                        rhs=xt_s[:, c, :],
                                        start=(c == 0),
                                        stop=(c == dck - 1),
                                    )
                                # evict + per-partition bias on ACT -> bf16
                                raw = pt.tile([PB, XSL], BF16, tag="raw",
                                              name="raw")
                                nc.scalar.activation(
                                    out=raw, in_=ps, func=AF.Identity,
                                    bias=bias_s[:, h:h + 1], scale=1.0,
                                )
                                # rope on DVE:
                                #   out = raw*cos + shift64(raw)*sin_folded
                                rs = pt.tile([PB, XSL], BF16, tag="rs",
                                             name="rs")
                                nc.vector.tensor_mul(
                                    rs[0:64], raw[64:128],
                                    sinT_s[64:128, ts])
                                nc.vector.tensor_mul(
                                    rs[64:128], raw[0:64], sinT_s[0:64, ts])
                                cq = pt.tile([PB, XSL], BF16, tag="cq",
                                             name="cq")
                                nc.vector.tensor_mul(cq, raw, cosT_s[:, ts])
                                nc.vector.tensor_add(outT[:, h, ts], cq, rs)
                            if sl == 0 and h + 1 < hpc:
                                load_w(h + 1, nc.scalar)
                            if sl == 0 and h == 2:
                                nc.gpsimd.dma_start(out=wv_h0,
                                                    in_=wv8.ap()[0])
                        for kind in ("q", "k"):
                            w_tiles.pop((kind, h))

                # ---- A2: v projection (natural layout, x slices reused) ---
                # nci-outer: the first half of wv covers 16 groups of
                # compute, so the second half streams in fully hidden.
                with (
                    tc.tile_pool(name="w_v1", bufs=1) as pwv1,
                    tc.tile_pool(name="psA2", bufs=3, space="PSUM") as psA,
                ):
                    wv_h1 = pwv1.tile([PB, dck, 512], BF16, tag="wv1",
                                      name="wv_h1")
                    nc.gpsimd.dma_start(out=wv_h1, in_=wv8.ap()[1])
                    for nci, wv_h in ((0, wv_h0), (1, wv_h1)):
                        ns = slice(nci * 512, (nci + 1) * 512)
                        for kcg in range(ntc128):
                            xt_s = xt_tiles[kcg // (XSL // PB)]
                            t128 = kcg % (XSL // PB)
                            ps = psA.tile([PB, 512], F32, tag="ps_v",
                                          name="ps_v")
                            for c in range(dck):
                                nc.tensor.matmul(
                                    ps,
                                    lhsT=xt_s[:, c,
                                              t128 * PB:(t128 + 1) * PB],
                                    rhs=wv_h[:, c, :],
                                    start=(c == 0),
                                    stop=(c == dck - 1),
                                )
                            nc.vector.tensor_copy(v_all[:, kcg, ns], ps)

            # ========== Phase B+C: attention fused with projection =========
            with (
                tc.tile_pool(name="ot", bufs=1) as po,
                tc.tile_pool(name="wp_p", bufs=1) as pwp,
                tc.tile_pool(name="bconsts", bufs=1) as bconsts,
            ):
                # mask constants (gpsimd affine_select used only here, at
                # the phase boundary: swapping gpsimd custom-op libraries
                # mid-loop costs ~us per swap).
                # ident/tri: matmul-side causal mask for the first block.
                # masks[r][p, f] = 1.0 iff f >= p + 128*r for the rest.
                ident = bconsts.tile([PB, PB], BF16, tag="ident")
                nc.vector.memset(ident, 1.0)
                nc.gpsimd.affine_select(
                    out=ident, in_=ident, compare_op=ALU.is_ge, fill=0.0,
                    base=0, pattern=[[1, PB]], channel_multiplier=-1)
                nc.gpsimd.affine_select(
                    out=ident, in_=ident, compare_op=ALU.is_ge, fill=0.0,
                    base=0, pattern=[[-1, PB]], channel_multiplier=1)
                tri = bconsts.tile([PB, 896], BF16, tag="tri")
                nc.vector.memset(tri, 0.0)
                nc.gpsimd.affine_select(
                    out=tri, in_=tri, compare_op=ALU.is_ge, fill=NEG_BIG,
                    base=-384, pattern=[[1, 896]], channel_multiplier=-1)
                masks = []
                for r in range(QB // PB):
                    mk_f = bconsts.tile([PB, QB], F32, tag="mask_f",
                                        name="mask_f")
                    nc.vector.memset(mk_f, 1.0)
                    nc.gpsimd.affine_select(
                        out=mk_f, in_=mk_f, compare_op=ALU.is_ge, fill=0.0,
                        base=-(r * PB), pattern=[[1, QB]],
                        channel_multiplier=-1,
                    )
                    mk = bconsts.tile([PB, QB], BF16, tag=f"mask{r}",
                                      name=f"mask{r}")
                    nc.vector.tensor_copy(mk, mk_f)
                    masks.append(mk)

                ot_all = po.tile([HD, hpc, t], BF16, tag="ot")
                # prefetch the output-projection weights on the gpsimd queue
                wp_s = pwp.tile([PB, hpc, d], BF16, tag="wp", name="wp_s")
                wp_src = wp.ap().rearrange("(h p) e -> p h e", p=PB)
                for h in range(hpc):
                    nc.gpsimd.dma_start(out=wp_s[:, h, :],
                                        in_=wp_src[:, h, :])
                with (
                    tc.tile_pool(name="pt_pool", bufs=6) as pp,
                    tc.tile_pool(name="zac", bufs=2) as pza,
                    tc.tile_pool(name="small", bufs=2) as psm,
                    tc.tile_pool(name="yout", bufs=3) as py,
                    tc.tile_pool(name="psS", bufs=2, space="PSUM") as psS,
                    tc.tile_pool(name="psO", bufs=1, space="PSUM") as psO,
                    tc.tile_pool(name="psZ", bufs=1, space="PSUM") as psZ,
                    tc.tile_pool(name="psY", bufs=2, space="PSUM") as psY,
                ):
                    # attention q-chunks in DESCENDING order, head-inner;
                    # the projection for q-chunk qc+1 interleaves with the
                    # attention pairs of q-chunk qc.
                    qcs = list(range(nqc - 1, -1, -1))
                    pairs = []
                    block_start = {}
                    for qc in qcs:
                        block_start[qc] = len(pairs)
                        for h in range(hpc):
                            for kcp in range((qc + 1) * kpq // 2):
                                pairs.append((qc, h, kcp))
                    st = {}

                    def emit_S(i):
                        qc, h, kcp = pairs[i]
                        if kcp == 0:
                            st[(qc, h, "o")] = psO.tile(
                                [HD, QB], F32, tag="ps_o", name="ps_o")
                            st[(qc, h, "z")] = pza.tile(
                                [PB, 2 * QB], BF16, tag="z_acc",
                                name="z_acc")
                        ps_s2 = psS.tile([PB, 2 * QB], F32, tag="ps_s",
                                         name="ps_s2")
                        qs = slice(qc * QB, (qc + 1) * QB)
                        for j in (0, 1):
                            kc = 2 * kcp + j
                            r = kc - qc * kpq
                            # first (filler-less) block: fold the causal
                            # mask into the S accumulation with a -BIG
                            # triangle matmul, so exp needs no gpsimd hop
                            mask_mm = (qc == qcs[0] and r >= 0)
                            nc.tensor.matmul(
                                ps_s2[:, j * QB:(j + 1) * QB],
                                lhsT=kT_all[:, h, kc * PB:(kc + 1) * PB],
                                rhs=qT_all[:, h, qs],
                                start=True, stop=not mask_mm,
                            )
                            if mask_mm:
                                nc.tensor.matmul(
                                    ps_s2[:, j * QB:(j + 1) * QB],
                                    lhsT=ident,
                                    rhs=tri[:, 384 - 128 * r:
                                            896 - 128 * r],
                                    start=False, stop=True,
                                )
                        st[i] = ps_s2

                    def emit_expmask(i):
                        qc, h, kcp = pairs[i]
                        ps_s2 = st.pop(i)
                        pt2 = pp.tile([PB, 2 * QB], BF16, tag="pt",
                                      name="pt2")
                        nc.scalar.activation(
                            out=pt2, in_=ps_s2, func=AF.Exp, scale=SCALE,
                        )
                        # causal mask: multiply diagonal blocks by the
                        # precomputed mask on DVE (the first block is
                        # already masked via the triangle matmul)
                        for j in (0, 1):
                            r = 2 * kcp + j - qc * kpq
                            if r >= 0 and qc != qcs[0]:
                                nc.vector.tensor_mul(
                                    pt2[:, j * QB:(j + 1) * QB],
                                    pt2[:, j * QB:(j + 1) * QB],
                                    masks[r],
                                )
                        st[(i, "pt")] = pt2

                    def emit_zo(i):
                        qc, h, kcp = pairs[i]
                        nkc = (qc + 1) * kpq
                        pt2 = st.pop((i, "pt"))
                        z_acc = st[(qc, h, "z")]
                        # softmax denominator: accumulate exp tiles on DVE
                        if kcp == 0:
                            nc.vector.tensor_copy(z_acc, pt2)
                        else:
                            nc.vector.tensor_add(z_acc, z_acc, pt2)
                        ps_o = st[(qc, h, "o")]
                        for j in (0, 1):
                            kc = 2 * kcp + j
                            nc.tensor.matmul(
                                ps_o,
                                lhsT=v_all[:, kc, h * HD:(h + 1) * HD],
                                rhs=pt2[:, j * QB:(j + 1) * QB],
                                start=(kc == 0), stop=(kc == nkc - 1),
                            )
                        if 2 * kcp + 1 == nkc - 1:
                            # head (h, qc) done: evict the O accumulator on
                            # ACT (frees the psO bank fast), then finish the
                            # softmax denominator in two deferred stages so
                            # no engine queue ever waits cross-engine:
                            #   stage 1 (next pair): PE column-sums z_acc
                            #     (the DVE z-adds have drained by then)
                            #   stage 2 (pair after): DVE reciprocal, then
                            #     gpsimd broadcast + normalize multiply
                            ot_tmp = pp.tile([HD, QB], BF16,
                                             tag="ot_tmp", name="ot_tmp",
                                             bufs=2)
                            nc.scalar.copy(ot_tmp, st.pop((qc, h, "o")))
                            z_fin = st.pop((qc, h, "z"))

                            def stage1(z_acc=z_fin, qc=qc, h=h):
                                ps_z = psZ.tile([1, QB], F32, tag="ps_z",
                                                name="ps_z")
                                for j in (0, 1):
                                    nc.tensor.matmul(
                                        ps_z,
                                        lhsT=ones_col,
                                        rhs=z_acc[:, j * QB:(j + 1) * QB],
                                        start=(j == 0), stop=(j == 1),
                                    )
                                st[(qc, h, "pz")] = ps_z

                            def stage2(qc=qc, h=h):
                                ps_z = st.pop((qc, h, "pz"))
                                rz = psm.tile([1, QB], F32, tag="rz",
                                              name="rz")
                                nc.vector.reciprocal_approx_fast(
                                    out=rz, in_=ps_z)
                                rzb = pp.tile([HD, QB], F32, tag="rzb",
                                              name="rzb", bufs=2)
                                nc.gpsimd.partition_broadcast(rzb, rz)
                                st[(qc, h, "rzb")] = rzb

                            def stage3(qc=qc, h=h, ot_tmp=ot_tmp):
                                qs = slice(qc * QB, (qc + 1) * QB)
                                nc.vector.tensor_mul(
                                    ot_all[:, h, qs], ot_tmp,
                                    st.pop((qc, h, "rzb")))

                            pending.append((i + 1, stage1))
                            pending.append((i + 2, stage2))
                            pending.append((i + 3, stage3))

                    def emit_c_group(cqc, gi):
                        t128 = cqc * kpq + gi // nec
                        nci = gi % nec
                        es = slice(nci * EC, (nci + 1) * EC)
                        if nci == 0:
                            st[("y", t128)] = py.tile(
                                [PB, d], BF16, tag="y_t", name="y_t")
                        y_t = st[("y", t128)]
                        ps_y = psY.tile([PB, EC], F32, tag="ps_y",
                                        name="ps_y")
                        for h in range(hpc):
                            nc.tensor.matmul(
                                ps_y,
                                lhsT=ot_all[:, h,
                                            t128 * PB:(t128 + 1) * PB],
                                rhs=wp_s[:, h, es],
                                start=(h == 0), stop=(h == hpc - 1),
                            )
                        nc.vector.tensor_copy(y_t[:, es], ps_y)
                        if cqc == qcs[-1]:
                            # final block: stream each column chunk so the
                            # last DMA before teardown is small
                            nc.sync.dma_start(
                                out=y.ap()[t128 * PB:(t128 + 1) * PB, es],
                                in_=y_t[:, es],
                            )
                            if nci == nec - 1:
                                st.pop(("y", t128))
                        elif nci == nec - 1:
                            nc.sync.dma_start(
                                out=y.ap()[t128 * PB:(t128 + 1) * PB, :],
                                in_=st.pop(("y", t128)),
                            )

                    ngroups = kpq * nec      # C matmul groups per q-chunk
                    emitted_c = {qc: 0 for qc in range(nqc)}
                    pending = []
                    emit_S(0)
                    emit_expmask(0)
                    emit_S(1)
                    for i in range(len(pairs)):
                        if i + 2 < len(pairs):
                            emit_S(i + 2)
                        if i + 1 < len(pairs):
                            emit_expmask(i + 1)
                        for due, fn in [p for p in pending if p[0] <= i]:
                            pending.remove((due, fn))
                            fn()
                        emit_zo(i)
                        qc = pairs[i][0]
                        bi = qcs.index(qc)
                        if bi >= 1:
                            cqc = qcs[bi - 1]   # project the previous block
                            j = i - block_start[qc]
                            npq = hpc * (qc + 1) * kpq // 2
                            # hold injection until the previous block's
                            # deferred finalize stages have been emitted
                            target = ((j - 3) * ngroups // npq
                                      if j >= 4 else 0)
                            while emitted_c[cqc] < target:
                                emit_c_group(cqc, emitted_c[cqc])
                                emitted_c[cqc] += 1
                    for due, fn in pending:
                        fn()
                    pending = []
                    for qc in qcs[:-1]:
                        while emitted_c[qc] < ngroups:
                            emit_c_group(qc, emitted_c[qc])
                            emitted_c[qc] += 1
                    for gi in range(ngroups):
                        emit_c_group(qcs[-1], gi)
    if compile:
        nc.compile()
    return nc


def make_in_maps(x, cos, sin, W_qkv, b_qkv, W_proj):
    """Host-side sharding: build the 8 per-core input dicts (bf16 casts)."""
    d = x.shape[-1]
    dck = d // PB
    in_maps = []
    cosT = np.ascontiguousarray(cos.reshape(-1, HD).T).astype(np.float32)
    sinT = np.ascontiguousarray(sin.reshape(-1, HD).T).astype(np.float32)
    sinTs = sinT.copy()
    sinTs[: HD // 2] = -sinTs[: HD // 2]
    sinTr = np.roll(sinTs, -(HD // 2), axis=0)
    cosT = cosT.astype(BF)
    sinTr = sinTr.astype(BF)
    Wq = np.asarray(W_qkv[:, 0 * d:1 * d], np.float32)
    Wk = np.asarray(W_qkv[:, 1 * d:2 * d], np.float32)
    Wv = np.asarray(W_qkv[:, 2 * d:3 * d], np.float32)

    def wqk8(Wm):
        # [d, hw] -> [h, p, c*m]  (contiguous per head for fast DMA)
        return np.ascontiguousarray(
            Wm.reshape(dck, PB, HPC, HD).transpose(2, 1, 0, 3)
        ).astype(BF).reshape(HPC, PB, dck * HD)

    def wv8(Wm):
        # [d, hw] -> [nci, p, c*m]
        return np.ascontiguousarray(
            Wm.reshape(dck, PB, 2, 512).transpose(2, 1, 0, 3)
        ).astype(BF).reshape(2, PB, dck * 512)

    def x8(xb):
        # x[b] [t, d] -> x.T [d, t] -> [sl, cc8, p, c8*tt]
        xT = np.asarray(xb, np.float32).T
        return np.ascontiguousarray(
            xT.reshape(2, 8, PB, 4, XSL).transpose(3, 0, 2, 1, 4)
        ).astype(BF).reshape(4, 2, PB, 8 * XSL)

    for c in range(N_CORES):
        b = c // 2
        g = c % 2
        hw = HPC * HD
        cs = slice(g * hw, (g + 1) * hw)
        in_maps.append(
            {
                "x8": x8(x[b]),
                "wq8": wqk8(Wq[:, cs]),
                "wk8": wqk8(Wk[:, cs]),
                "wv8": wv8(Wv[:, cs]),
                "bq": np.ascontiguousarray(b_qkv[0 * d:1 * d][cs], np.float32),
                "bk": np.ascontiguousarray(b_qkv[1 * d:2 * d][cs], np.float32),
                "wp": np.ascontiguousarray(
                    np.asarray(W_proj, np.float32)[g * hw:(g + 1) * hw, :]
                ).astype(BF),
                "cosT": cosT,
                "sinTr": sinTr,
            }
        )
    return in_maps


def gather_output(results, b_qkv, W_proj, b_proj):
    """Sum the per-core partials and add the bias terms."""
    d = W_proj.shape[1]
    # v-bias contributes (sum_k attn = 1) exactly b_v @ W_proj per token.
    host_bias = (
        np.asarray(b_qkv[2 * d:3 * d], np.float32)
        @ np.asarray(W_proj, np.float32)
        + np.asarray(b_proj, np.float32)
    )
    y = np.empty((B, T, d), np.float32)
    for b in range(B):
        y[b] = (np.asarray(results[2 * b]["y"], np.float32)
                + np.asarray(results[2 * b + 1]["y"], np.float32)
                + host_bias)
    return y


_NC_CACHE = {}


def kernel(x, cos, sin, W_qkv, b_qkv, W_proj, b_proj):
    x = np.asarray(x, np.float32)
    key = "full"
    if key not in _NC_CACHE:
        _NC_CACHE[key] = build_nc()
    nc = _NC_CACHE[key]
    in_maps = make_in_maps(
        x,
        np.asarray(cos, np.float32),
        np.asarray(sin, np.float32),
        np.asarray(W_qkv, np.float32),
        np.asarray(b_qkv, np.float32),
        np.asarray(W_proj, np.float32),
    )
    res = run_bass_kernel_spmd(nc, in_maps, core_ids=list(range(N_CORES)))
    return gather_output(res.results, b_qkv, W_proj, b_proj)


if __name__ == "__main__":
    import reference

    inputs = reference.setup_inputs()
    out = kernel(**{k: np.asarray(v) for k, v in inputs.items()})
    exp = np.asarray(reference.reference(**inputs))
    err = np.abs(out - exp).max() / np.abs(exp).max()
    print("rel err:", err)
